# revision 6
# baseline (speedup 1.0000x reference)
"""Canny filter Bass kernel for Trainium2, data-parallel over batch on 8 cores.

v3: full 3x3 Sobel convolutions (vertical band x horizontal shift) run on the
tensor engine as column-shifted accumulating float32r matmuls over zero-padded
tiles, with hi/lo input splitting for exactness; the hysteresis 3x3 sum also
runs fully on the PE (3-shift T3).  DVE keeps only the non-linear work:
orientation, NMS maxes/selection, fused threshold/hysteresis custom ops.
"""

import os
from contextlib import ExitStack

import numpy as np
import ml_dtypes

import concourse.bacc as bacc
import concourse.tile as tile
from concourse import mybir
from concourse.bass_utils import run_bass_kernel_spmd

F32 = mybir.dt.float32
F32R = mybir.dt.float32r
I32 = mybir.dt.int32
U8 = mybir.dt.uint8
BF16 = mybir.dt.bfloat16
AF = mybir.ActivationFunctionType
ALU = mybir.AluOpType

H = W = 1024
C = 3
NB = 8          # row blocks
P = 128         # rows per block
HALF = 512      # fp32 matmul max moving free dim
WP = W + 2      # padded width
INV3 = float(np.float32(1.0) / np.float32(3.0))
INV9 = float(np.float32(INV3) * np.float32(INV3))
K8PI = float(np.float32(8.0 / np.pi))

DBG = int(os.environ.get("KDBG", "9"))

# ---------------------------------------------------------------------------
# Custom DVE ops (registered into the concourse dve_ops registry).
# ---------------------------------------------------------------------------
from concourse import dve_ops as _dvo
from concourse.dve_spec import Spec, Src0, Src1, sq, maxx, lower, _has_src1
from concourse.dve_spec import C0 as _C0, C1 as _C1, C2 as _C2
from concourse.dve_uop import DveOpSpec


def _register_op(name, body, reference):
    if name in _dvo._SUB_OPCODE_FOR_NAME:
        for op in _dvo.OPS:
            if op.name == name:
                return op
    spec = Spec(body=body, reference=reference)
    row = max(_dvo._SUB_OPCODE_FOR_NAME.values()) + 1
    assert row < 0x20, "custom DVE opcode rows exhausted"
    _dvo._SUB_OPCODE_FOR_NAME[name] = row
    shas = {}
    for ver in ("v3", "v4"):
        uops = lower(spec, ver=ver)
        shas[ver] = DveOpSpec(
            name=name, opcode=row, uops=uops, rd1_en=_has_src1(spec)
        ).sha(ver)
    op = _dvo.DveOp(name, spec, subdim=False, uops_sha=shas)
    _dvo.OPS.append(op)
    _dvo.CUSTOM_DVE_SPECS[name] = spec
    return op


# q = (gx^2 + gy^2) * c0   (c0 = 1/9 folds the /C channel normalization)
QSQ = _register_op(
    "CANNY_QSQ_ANT",
    (sq(Src0) + sq(Src1)) * _C0,
    lambda in0, in1, s0, s1, imm2: (
        (in0.astype(np.float32) ** 2 + in1.astype(np.float32) ** 2) * s0
    ).astype(np.float32),
)

# bt = (q > max(M, c0)) + (q > max(M, c1))   (c0=low^2, c1=high^2)
BTQ = _register_op(
    "CANNY_BTQ_ANT",
    (Src0 > maxx(Src1, _C0)) + (Src0 > maxx(Src1, _C1)),
    lambda in0, in1, s0, s1, imm2: (
        (in0 > np.maximum(in1, s0)).astype(np.float32)
        + (in0 > np.maximum(in1, s1)).astype(np.float32)
    ),
)

# fin = hi + (lo_any - hi) * (S > c2); hi = bt > c1, lo_any = bt > c0
_hi = Src0 > _C1
FIN = _register_op(
    "CANNY_FIN_ANT",
    _hi + ((Src0 > _C0) - _hi) * (Src1 > _C2),
    lambda in0, in1, s0, s1, imm2: (
        (in0 > s1).astype(np.float32)
        + ((in0 > s0).astype(np.float32) - (in0 > s1).astype(np.float32))
        * (in1 > imm2).astype(np.float32)
    ),
)

# o1 = max(min(Src0*c0 + c1, c2), 0) -> i32 (rounds on output convert);
# DVE max(NaN, 0) = 0, so garbage arctan inputs land in-range
from concourse.dve_spec import minn as _minn, Zero as _Zero
OCLAMP = _register_op(
    "CANNY_OCLAMP_ANT",
    maxx(_minn(Src0 * _C0 + _C1, _C2), _Zero),
    lambda in0, in1, s0, s1, imm2: np.maximum(
        np.minimum(in0.astype(np.float32) * s0 + s1, imm2), 0.0
    ).astype(np.float32),
)

# f32r weight block ids
(W_VS, W_VSM, W_VD, W_VDH, W_SUP, W_SDN,
 W_VSP, W_VSPM, W_VSN, W_VSNM,
 W_VDP, W_VDPH, W_VDN, W_VDNH,
 W_SUPN, W_SDNP) = range(16)


def _const_weights():
    """f32 [128, 16*128] f32r-exact weight blocks (see W_* ids).

    Vs: vertical [0.5,1,0.5]; VsM = -Vs; Vd: vertical [-1,0,1] (row r-1
    weight -1); VdH = Vd/2; Sup: out[r]=in[r+1]; Sdn: out[r]=in[r-1].
    *P blocks map the PREV block's row 127 to out row 0 (w[127,0]);
    *N blocks map the NEXT block's row 0 to out row 127 (w[0,127]).
    """
    cw = np.zeros((P, 16 * P), np.float32)

    def blk(i):
        return cw[:, i * P:(i + 1) * P]

    Vs, Vd = blk(W_VS), blk(W_VD)
    Sup, Sdn = blk(W_SUP), blk(W_SDN)
    for m in range(P):
        Vs[m, m] = 1.0
        if m > 0:
            Vs[m - 1, m] = 0.5
            Vd[m - 1, m] = -1.0
            Sdn[m - 1, m] = 1.0
        if m < P - 1:
            Vs[m + 1, m] = 0.5
            Vd[m + 1, m] = 1.0
            Sup[m + 1, m] = 1.0
    blk(W_VSM)[:] = -Vs
    blk(W_VDH)[:] = 0.5 * Vd
    blk(W_VSP)[P - 1, 0] = 0.5
    blk(W_VSPM)[P - 1, 0] = -0.5
    blk(W_VSN)[0, P - 1] = 0.5
    blk(W_VSNM)[0, P - 1] = -0.5
    blk(W_VDP)[P - 1, 0] = -1.0
    blk(W_VDPH)[P - 1, 0] = -0.5
    blk(W_VDN)[0, P - 1] = 1.0
    blk(W_VDNH)[0, P - 1] = 0.5
    blk(W_SUPN)[0, P - 1] = 1.0
    blk(W_SDNP)[P - 1, 0] = 1.0
    return cw


def _const_weights_bf16():
    """bf16 [128, 3*128]: T3 vertical [1,1,1] | T3P | T3N halo matrices."""
    cwb = np.zeros((P, 3 * P), np.float32)
    t3 = cwb[:, 0:P]
    for m in range(P):
        t3[m, m] = 1.0
        if m > 0:
            t3[m - 1, m] = 1.0
        if m < P - 1:
            t3[m + 1, m] = 1.0
    cwb[P - 1, P] = 1.0          # T3P
    cwb[0, 3 * P - 1] = 1.0      # T3N
    return cwb.astype(ml_dtypes.bfloat16)


def _emit(nc, tc, img, cw, cwb, o_gx, o_gy, o_q, o_or, o_te):
    v = nc.vector
    sc = nc.scalar
    te = nc.tensor
    gp = nc.gpsimd

    ctx = ExitStack()
    cpool = ctx.enter_context(tc.tile_pool(name="cp", bufs=1))
    inp = ctx.enter_context(tc.tile_pool(name="inp", bufs=2))
    spool = ctx.enter_context(tc.tile_pool(name="sp", bufs=2))
    shpool = ctx.enter_context(tc.tile_pool(name="shp", bufs=3))
    slpool = ctx.enter_context(tc.tile_pool(name="slp", bufs=3))
    sb1 = ctx.enter_context(tc.tile_pool(name="sb1", bufs=1))
    nms2 = ctx.enter_context(tc.tile_pool(name="nms2", bufs=2))
    qpool = ctx.enter_context(tc.tile_pool(name="qp", bufs=2))
    qhpool = ctx.enter_context(tc.tile_pool(name="qhp", bufs=3))
    btpool = ctx.enter_context(tc.tile_pool(name="btp", bufs=4))
    mpool = ctx.enter_context(tc.tile_pool(name="mp", bufs=2))
    outp = ctx.enter_context(tc.tile_pool(name="outp", bufs=2))
    psGA = ctx.enter_context(tc.tile_pool(name="psGA", bufs=1, space="PSUM"))
    psGB = ctx.enter_context(tc.tile_pool(name="psGB", bufs=1, space="PSUM"))
    psS = ctx.enter_context(tc.tile_pool(name="psS", bufs=1, space="PSUM"))

    cwt = cpool.tile([P, 16 * P], F32R, tag="cw")
    nc.sync.dma_start(cwt[:], cw[:])
    cwbt = cpool.tile([P, 3 * P], BF16, tag="cwb")
    nc.sync.dma_start(cwbt[:], cwb[:])

    def wblk(i):
        return cwt[:, i * P:(i + 1) * P]

    T3 = cwbt[:, 0:P]
    T3P = cwbt[:, P:2 * P]
    T3N = cwbt[:, 2 * P:3 * P]

    def sconv(out_ps, parts):
        """Accumulate shifted matmuls: parts = [(w, padded_tensor, dcol)].

        Tensors are [P, W+2] zero-padded; out is [P, W] PSUM.  All matmuls
        cover the full 512-col half (pads make shifts always in range).
        """
        for h in (0, HALF):
            n = len(parts)
            for i, (wt, tp, d) in enumerate(parts):
                rh = tp[:, h + 1 + d:h + 1 + d + HALF]
                te.matmul(out_ps[:, h:h + HALF], wt, rh,
                          start=(i == 0), stop=(i == n - 1))

    s_hi = [None] * NB
    s_lo = [None] * NB
    q_sb = [None] * NB
    q_hi = [None] * NB
    bt_sb = [None] * NB
    m_sb = [None] * NB

    for it in range(NB + 3):
        # ---------------- stage 0: load + channel sum (exact) --------------
        b = it
        if b < NB:
            xt = inp.tile([P, C * W], F32, tag="x")
            for c in range(C):
                nc.sync.dma_start(xt[:, c * W:(c + 1) * W],
                                  img[c, b * P:(b + 1) * P, :])
            s01 = sb1.tile([P, W], F32, tag="s01")
            gp.tensor_tensor(s01[:], xt[:, 0:W], xt[:, W:2 * W], ALU.add)
            st = spool.tile([P, W], F32, tag="s")
            gp.tensor_tensor(st[:], s01[:], xt[:, 2 * W:3 * W], ALU.add)
            sh = shpool.tile([P, WP], F32R, tag="sh")
            s_hi[b] = sh
            gp.memset(sh[:, 0:1].bitcast(F32), 0.0)
            gp.memset(sh[:, WP - 1:WP].bitcast(F32), 0.0)
            sc.activation(sh[:, 1:W + 1], st[:], AF.Copy)
            sl = slpool.tile([P, WP], F32R, tag="sl")
            s_lo[b] = sl
            gp.memset(sl[:, 0:1].bitcast(F32), 0.0)
            gp.memset(sl[:, WP - 1:WP].bitcast(F32), 0.0)
            v.tensor_tensor(sl[:, 1:W + 1], st[:],
                            sh[:, 1:W + 1].bitcast(F32), ALU.subtract)

        # ---------------- stage 1: gradients, q, orientation ----------------
        j = it - 1
        if 0 <= j < NB and DBG >= 2:
            prev = s_hi[j - 1] if j > 0 else None
            nxt = s_hi[j + 1] if j < NB - 1 else None
            # gx = t[c+1] - t[c-1], t = Vs . s  (all on PE)
            ps_gx = psGA.tile([P, W], F32, tag="gA")
            parts = [(wblk(W_VS), s_hi[j], +1), (wblk(W_VSM), s_hi[j], -1),
                     (wblk(W_VS), s_lo[j], +1), (wblk(W_VSM), s_lo[j], -1)]
            if prev is not None:
                parts += [(wblk(W_VSP), prev, +1), (wblk(W_VSPM), prev, -1)]
            if nxt is not None:
                parts += [(wblk(W_VSN), nxt, +1), (wblk(W_VSNM), nxt, -1)]
            sconv(ps_gx, parts)
            gxs = sb1.tile([P, W], F32, tag="gxs")
            sc.activation(gxs[:], ps_gx[:], AF.Copy)

            # gy = 0.5 u[c-1] + u[c] + 0.5 u[c+1], u = Vd . s  (all on PE)
            ps_gy = psGB.tile([P, W], F32, tag="gB")
            parts = [(wblk(W_VD), s_hi[j], 0), (wblk(W_VD), s_lo[j], 0),
                     (wblk(W_VDH), s_hi[j], +1), (wblk(W_VDH), s_lo[j], +1),
                     (wblk(W_VDH), s_hi[j], -1), (wblk(W_VDH), s_lo[j], -1)]
            if prev is not None:
                parts += [(wblk(W_VDP), prev, 0), (wblk(W_VDPH), prev, +1),
                          (wblk(W_VDPH), prev, -1)]
            if nxt is not None:
                parts += [(wblk(W_VDN), nxt, 0), (wblk(W_VDNH), nxt, +1),
                          (wblk(W_VDNH), nxt, -1)]
            sconv(ps_gy, parts)

            gxo = outp.tile([P, W], BF16, tag="gxo")
            sc.activation(gxo[:], gxs[:], AF.Copy, scale=INV3)
            nc.sync.dma_start(o_gx[j * P:(j + 1) * P, :], gxo[:])
            gyo = outp.tile([P, W], BF16, tag="gyo")
            sc.activation(gyo[:], ps_gy[:], AF.Copy, scale=INV3)
            nc.sync.dma_start(o_gy[j * P:(j + 1) * P, :], gyo[:])

            # q = (gx^2 + gy^2) / 9, zero-padded one col each side
            q = qpool.tile([P, WP], F32, tag="q")
            gp.memset(q[:, 0:1], 0.0)
            gp.memset(q[:, W + 1:W + 2], 0.0)
            v._custom_dve(QSQ, out=q[:, 1:W + 1], in0=gxs[:], in1=ps_gy[:],
                          s0=INV9)
            q_sb[j] = q
            qh = qhpool.tile([P, WP], F32R, tag="qh")
            q_hi[j] = qh
            gp.memset(qh[:, 0:1].bitcast(F32), 0.0)
            gp.memset(qh[:, WP - 1:WP].bitcast(F32), 0.0)
            sc.activation(qh[:, 1:W + 1], q[:, 1:W + 1], AF.Copy)
            qb = outp.tile([P, W], BF16, tag="qb")
            sc.activation(qb[:], q[:, 1:W + 1], AF.Copy)
            nc.sync.dma_start(o_q[j * P:(j + 1) * P, :], qb[:])

            if DBG < 3:
                continue
            # orientation: r = gy/gx; o1 = clamp(round(arctan(r)*8/pi + 4))
            rv = sb1.tile([P, W], F32, tag="rv")
            v.reciprocal_approx_fast(rv[:], gxs[:])
            r = sb1.tile([P, W], F32, tag="r")
            v.tensor_tensor(r[:], ps_gy[:], rv[:], ALU.mult)
            arct = sb1.tile([P, W], F32, tag="arct")
            sc.activation(arct[:], r[:], AF.Arctan)
            o1i = sb1.tile([P, W], I32, tag="o1i")
            v._custom_dve(OCLAMP, out=o1i[:], in0=arct[:], s0=K8PI, s1=4.0,
                          imm2=8.0)
            oro = outp.tile([P, W], U8, tag="oro")
            gp.tensor_copy(oro[:], o1i[:])
            nc.sync.dma_start(o_or[j * P:(j + 1) * P, :], oro[:])
            pi_ = sb1.tile([P, W], I32, tag="pi")
            v.tensor_scalar(pi_[:], o1i[:], 3, None, ALU.bitwise_and)
            ms = mpool.tile([P, 3 * W], U8, tag="m")
            for mi in (1, 2, 3):
                gp.tensor_scalar(ms[:, (mi - 1) * W:mi * W], pi_[:], mi, None,
                                 ALU.is_equal)
            m_sb[j] = ms

        # ---------------- stage 2: NMS + thresholds ----------------
        k = it - 2
        if 0 <= k < NB and DBG >= 4:
            q = q_sb[k]
            nxt_q = q_hi[k + 1] if k < NB - 1 else None
            prev_q = q_hi[k - 1] if k > 0 else None
            ps_A = psGA.tile([P, W], F32, tag="gA")
            parts = [(wblk(W_SUP), q_hi[k], 0)]
            if nxt_q is not None:
                parts.append((wblk(W_SUPN), nxt_q, 0))
            sconv(ps_A, parts)
            ps_B = psGB.tile([P, W], F32, tag="gB")
            parts = [(wblk(W_SDN), q_hi[k], 0)]
            if prev_q is not None:
                parts.append((wblk(W_SDNP), prev_q, 0))
            sconv(ps_B, parts)
            qd = nms2.tile([P, W], F32, tag="qd")
            sc.activation(qd[:], ps_B[:], AF.Copy)

            M0 = nms2.tile([P, W], F32, tag="M0")
            v.tensor_tensor(M0[:], q[:, 0:W], q[:, 2:W + 2], ALU.max)
            M2 = nms2.tile([P, W], F32, tag="M2")
            v.tensor_tensor(M2[:], ps_A[:], qd[:], ALU.max)
            M1 = nms2.tile([P, W], F32, tag="M1")
            v.tensor_tensor(M1[:, 1:W - 1], ps_A[:, 2:W], qd[:, 0:W - 2],
                            ALU.max)
            v.tensor_copy(M1[:, 0:1], ps_A[:, 1:2])
            v.tensor_copy(M1[:, W - 1:W], qd[:, W - 2:W - 1])
            M3 = nms2.tile([P, W], F32, tag="M3")
            v.tensor_tensor(M3[:, 1:W - 1], ps_A[:, 0:W - 2], qd[:, 2:W],
                            ALU.max)
            v.tensor_copy(M3[:, 0:1], qd[:, 1:2])
            v.tensor_copy(M3[:, W - 1:W], ps_A[:, W - 2:W - 1])

            # with Sup = row-below / Sdn = row-above, the (A_r,B_l) max is
            # class 3's neighbor pair and (A_l,B_r) is class 1's
            ms = m_sb[k]
            v.copy_predicated(M0[:], ms[:, 0:W], M3[:])
            v.copy_predicated(M0[:], ms[:, W:2 * W], M2[:])
            v.copy_predicated(M0[:], ms[:, 2 * W:3 * W], M1[:])

            bt = btpool.tile([P, WP], BF16, tag="bt")
            bt_sb[k] = bt
            gp.memset(bt[:, 0:1], 0.0)
            gp.memset(bt[:, WP - 1:WP], 0.0)
            v._custom_dve(BTQ, out=bt[:, 1:W + 1], in0=q[:, 1:W + 1],
                          in1=M0[:], s0=0.25, s1=1.0)

        # ---------------- stage 3: 3x3 hysteresis sum on PE + fin ----------
        f = it - 3
        if 0 <= f < NB and DBG >= 5:
            bt = bt_sb[f]
            prev_c = bt_sb[f - 1] if f > 0 else None
            next_c = bt_sb[f + 1] if f < NB - 1 else None
            ps_S = psS.tile([P, W], F32, tag="S")
            parts = [(T3, bt, 0), (T3, bt, +1), (T3, bt, -1)]
            if prev_c is not None:
                parts += [(T3P, prev_c, 0), (T3P, prev_c, +1),
                          (T3P, prev_c, -1)]
            if next_c is not None:
                parts += [(T3N, next_c, 0), (T3N, next_c, +1),
                          (T3N, next_c, -1)]
            sconv(ps_S, parts)
            fin = outp.tile([P, W], BF16, tag="fin")
            v._custom_dve(FIN, out=fin[:], in0=bt[:, 1:W + 1], in1=ps_S[:],
                          s0=0.5, s1=1.5, imm2=1.5)
            nc.sync.dma_start(o_te[f * P:(f + 1) * P, :], fin[:])

    ctx.close()


def _build():
    nc = bacc.Bacc()
    img = nc.declare_dram_parameter("img", [C, H, W], F32, isOutput=False)
    cw = nc.declare_dram_parameter("cw", [P, 16 * P], F32R, isOutput=False)
    cwb = nc.declare_dram_parameter("cwb", [P, 3 * P], BF16, isOutput=False)
    o_gx = nc.declare_dram_parameter("o_gx", [H, W], BF16, isOutput=True)
    o_gy = nc.declare_dram_parameter("o_gy", [H, W], BF16, isOutput=True)
    o_q = nc.declare_dram_parameter("o_q", [H, W], BF16, isOutput=True)
    o_or = nc.declare_dram_parameter("o_or", [H, W], U8, isOutput=True)
    o_te = nc.declare_dram_parameter("o_te", [H, W], BF16, isOutput=True)
    with tile.TileContext(nc) as tc:
        _emit(nc, tc, img, cw, cwb, o_gx, o_gy, o_q, o_or, o_te)
    nc.finalize()
    return nc


_NC_CACHE = None


def _get_nc():
    global _NC_CACHE
    if _NC_CACHE is None:
        _NC_CACHE = _build()
    return _NC_CACHE


LAST_EXEC_TIME_NS = None


def kernel(img: np.ndarray):
    global LAST_EXEC_TIME_NS
    img = np.asarray(img, np.float32)
    B = img.shape[0]
    cw = _const_weights()
    cwb = _const_weights_bf16()
    nc = _get_nc()
    in_maps = [{"img": np.ascontiguousarray(img[i]), "cw": cw, "cwb": cwb}
               for i in range(B)]
    trace = bool(int(os.environ.get("KTRACE", "0")))
    out = run_bass_kernel_spmd(nc, in_maps, list(range(B)), trace=trace)
    if out.exec_time_ns is not None:
        LAST_EXEC_TIME_NS = out.exec_time_ns
    res = out.results
    gx = np.stack([res[i]["o_gx"] for i in range(B)])[:, None].astype(np.float32)
    gy = np.stack([res[i]["o_gy"] for i in range(B)])[:, None].astype(np.float32)
    q = np.stack([res[i]["o_q"] for i in range(B)])[:, None].astype(np.float32)
    gm = np.sqrt(q)
    o1 = np.stack([res[i]["o_or"] for i in range(B)])[:, None]
    orient = o1.astype(np.float32) * 45.0
    edges = np.stack([res[i]["o_te"] for i in range(B)])[:, None].astype(np.float32)
    return (gx, gy, gm, orient, edges)
